# revision 12
# baseline (speedup 1.0000x reference)
"""Trainium2 Bass kernel for nn_BaseGraph_67697274519895 (gnn_message_passing).

Reference computation (B=8, N=256, D=128, E=65280):
    edge_feat = concat([x[:, recv, :], x[:, send, :]], -1)        # [B, E, 2D]
    out = zeros([B, N, 2D]).at[:, recv, :].add(edge_feat) / N

With R/S the one-hot [E, N] incidence matrices of recv/send, the scatter-add
is out = R^T @ concat(R @ x, S @ x) / N, which collapses algebraically:
    out[:, :, :D]  = (R^T R) @ x / N = diag(bincount(recv)) @ x / N
    out[:, :, D:]  = (R^T S) @ x / N = A @ x / N,  A[i, j] = #edges (r=i, s=j)
Valid for arbitrary index arrays. A and the counts are built host-side from
the indices (O(E) bincount); the device runs, per batch element, a
[N, N] @ [N, D] matmul plus a per-node row scale.

Sharding: data-parallel over batch — core b handles x[b]; A and counts are
replicated to all 8 cores. No collectives.

Precision: A^T/N entries are small integer counts / 2^8 — EXACTLY
representable in bf16.  x is split host-side into bf16 hi + lo with
x = hi + lo to ~2^-18 relative; the device accumulates
    psum[d, n] = sum_k (hi_k^T + lo_k^T) @ (A^T)_k
in one fp32 PSUM group (bf16 x bf16 products are exact in fp32), matching a
full-fp32 matmul to ~4e-6 while running the PE 4x faster (1 cycle/row).
The x*cnt half is (hi*cnt + lo*cnt) for block 0 (pure bf16 operands, fp32
arithmetic) and x_f32*cnt for block 1.

DMA layout (tuned against the TRN2 cost model: per-DMA fixed costs and the
serial HWDGE/DMA-engine devices dominate at this size):
  - in0 [128, 386 f32 words] (1544B rows): hi0|lo0|A^T_0|x1_f32|cnt0|cnt1 —
    everything PE needs for k=0 and everything DVE needs for out1.
  - in1 [128, 256 words] (1024B rows): hi1|lo1|A^T_1 — k=1 matmul operands
    only, so the second (HWDGE-serialized) DMA stays light.
  - out o1 via one [128,2,128] tile (2 DMAs total out: o1 then o2t; o1's
    transfer hides under o2t's descriptor generation).
"""

import numpy as np

B, N, D = 8, 256, 128
N_CORES = 8
P = 128

# in0 word layout
IN0_HI = 0  # 64 words: hi0 (128 bf16)
IN0_LO = 64  # 64 words: lo0
IN0_AT = 128  # 128 words: A^T_0 (256 bf16)
IN0_X1 = 256  # 128 words: x1 f32
IN0_C0 = 384  # cnt0
IN0_C1 = 385  # cnt1
W0 = 386
# in1 word layout
IN1_HI = 0
IN1_LO = 64
IN1_AT = 128
W1 = 256

_PROGRAM = None


def _build_program():
    import concourse.mybir as mybir
    from concourse import bacc
    from concourse.tile import TileContext

    f32 = mybir.dt.float32
    bf16 = mybir.dt.bfloat16
    nc = bacc.Bacc(trn_type="TRN2")

    in0 = nc.dram_tensor("in0", [P, W0], f32, kind="ExternalInput")
    in1 = nc.dram_tensor("in1", [P, W1], f32, kind="ExternalInput")
    o1 = nc.dram_tensor("o1", [P, 2, D], f32, kind="ExternalOutput")
    o2t = nc.dram_tensor("o2t", [D, N], f32, kind="ExternalOutput")

    with TileContext(nc) as tc:
        with (
            tc.tile_pool(name="sbuf", bufs=1) as pool,
            tc.tile_pool(name="psum", bufs=1, space="PSUM") as psum_pool,
        ):
            t0 = pool.tile([P, W0], f32, name="t0")
            nc.sync.dma_start(out=t0[:], in_=in0[:])
            # in1 via the Pool-engine SWDGE path: its descriptor generation runs
            # on the Pool engine, in parallel with in0's on the (serial) HWDGE
            t1 = pool.tile([P, W1], f32, name="t1")
            nc.gpsimd.dma_start(out=t1[:], in_=in1[:])

            # psum[d, n] = sum_k (hi_k + lo_k)^T @ (A^T)_k
            ps = psum_pool.tile([P, N], f32, name="ps")
            at0 = t0[:, IN0_AT:IN0_X1].bitcast(bf16)
            at1 = t1[:, IN1_AT:W1].bitcast(bf16)
            mms = [
                (t0[:, IN0_HI:IN0_LO].bitcast(bf16), at0),
                (t0[:, IN0_LO:IN0_AT].bitcast(bf16), at0),
                (t1[:, IN1_HI:IN1_LO].bitcast(bf16), at1),
                (t1[:, IN1_LO:IN1_AT].bitcast(bf16), at1),
            ]
            for i, (lhsT, rhs) in enumerate(mms):
                nc.tensor.matmul(
                    ps[:], lhsT, rhs, start=(i == 0), stop=(i == len(mms) - 1)
                )

            # out1 block 0: (hi0 + lo0) * cnt0 in fp32; block 1: x1_f32 * cnt1
            ot1 = pool.tile([P, 2, D], f32, name="ot1")
            tmp = pool.tile([P, D], f32, name="tmp")
            nc.vector.tensor_scalar_mul(
                ot1[:, 0, :], t0[:, IN0_HI:IN0_LO].bitcast(bf16), t0[:, IN0_C0 : IN0_C0 + 1]
            )
            nc.vector.tensor_scalar_mul(
                tmp[:], t0[:, IN0_LO:IN0_AT].bitcast(bf16), t0[:, IN0_C0 : IN0_C0 + 1]
            )
            nc.vector.tensor_add(ot1[:, 0, :], ot1[:, 0, :], tmp[:])
            nc.vector.tensor_scalar_mul(
                ot1[:, 1, :], t0[:, IN0_X1:IN0_C0], t0[:, IN0_C1 : IN0_C1 + 1]
            )

            nc.sync.dma_start(out=o1[:], in_=ot1[:])
            ot2 = pool.tile([P, N], f32, name="ot2")
            nc.vector.tensor_copy(ot2[:], ps[:])
            nc.sync.dma_start(out=o2t[:], in_=ot2[:])

    nc.compile()
    return nc


def kernel(x, receivers, senders):
    global _PROGRAM
    import ml_dtypes
    from concourse.bass_utils import run_bass_kernel_spmd

    x = np.ascontiguousarray(np.asarray(x), dtype=np.float32)
    recv = np.asarray(receivers).astype(np.int64).ravel()
    send = np.asarray(senders).astype(np.int64).ravel()
    assert x.shape == (B, N, D), x.shape
    assert recv.min() >= 0 and recv.max() < N, (recv.min(), recv.max())
    assert send.min() >= 0 and send.max() < N, (send.min(), send.max())

    # A^T[s, r] = #edges with (receiver=r, sender=s); scaled by 1/N (exact, N=2^8)
    atc = (
        np.bincount(send * N + recv, minlength=N * N)
        .reshape(N, N)
        .astype(np.float32)
        / N
    )
    cnt = np.bincount(recv, minlength=N).astype(np.float32) / N

    bf = ml_dtypes.bfloat16
    xh = x.astype(bf)
    xl = (x - xh.astype(np.float32)).astype(bf)

    def words(a16):
        """bf16 array [..., 2k] -> f32 words [..., k]."""
        return np.ascontiguousarray(a16.view(np.uint16)).view(np.uint32).view(np.float32)

    xh_w = words(xh).reshape(B, 2, P, D // 2)
    xl_w = words(xl).reshape(B, 2, P, D // 2)
    at_w = words(atc.astype(bf)).reshape(2, P, N // 2)
    cnt2 = cnt.reshape(2, P)

    in0 = np.empty((B, P, W0), dtype=np.float32)
    in0[:, :, IN0_HI:IN0_LO] = xh_w[:, 0]
    in0[:, :, IN0_LO:IN0_AT] = xl_w[:, 0]
    in0[:, :, IN0_AT:IN0_X1] = at_w[0][None]
    in0[:, :, IN0_X1:IN0_C0] = x.reshape(B, 2, P, D)[:, 1]
    in0[:, :, IN0_C0] = cnt2[0][None]
    in0[:, :, IN0_C1] = cnt2[1][None]

    in1 = np.empty((B, P, W1), dtype=np.float32)
    in1[:, :, IN1_HI:IN1_LO] = xh_w[:, 1]
    in1[:, :, IN1_LO:IN1_AT] = xl_w[:, 1]
    in1[:, :, IN1_AT:W1] = at_w[1][None]

    if _PROGRAM is None:
        _PROGRAM = _build_program()
    nc = _PROGRAM

    in_maps = [{"in0": in0[b], "in1": in1[b]} for b in range(B)]
    res = run_bass_kernel_spmd(nc, in_maps, core_ids=list(range(N_CORES)))

    out = np.empty((B, N, 2 * D), dtype=np.float32)
    for b in range(B):
        r = res.results[b]
        # o1[p, k, :] holds row 128k+p of x*cnt/N
        out[b, :, 0:D] = r["o1"].transpose(1, 0, 2).reshape(N, D)
        # o2t[d, n] = (A @ x / N)[n, d]
        out[b, :, D : 2 * D] = r["o2t"].T
    return out
